# revision 26
# baseline (speedup 1.0000x reference)
"""Trainium2 Bass kernel for InvariantMessage GNN message passing.

out[e, :] = (MLP(s_j)[nbrs[e,1]]) * ((rbf(dist[e]) @ W_rbf + b_rbf) * env(dist[e]))

The axon tunnel (~30-90 MB/s) dominates wall time, so the design minimizes
host<->device bytes; on-device compute is ~0.1 s.

Strategy (8 cores, node-sharded):
- Nodes are split 6250/core; each EDGE is assigned to the core that owns its
  gathered node nbrs[e,1], so every gather is core-local (no collectives) and
  s_j is uploaded exactly once across the fleet (fp16, 1.7 MB/core).
- Each core runs the MLP on its 6250 nodes into an Internal-DRAM table
  (no zero upload), then per 3072-edge chunk gathers phi rows with
  indirect_dma_start (128 rows/instruction -- the HW-validated semantics).
- dist is uploaded once in a [12,128]-per-1536-edge layout; the quadrant-
  packed sin-argument tile is built on device by K=3 outer-product matmuls
  against a coefficient matrix (row n<20: (n+1)/10, row 20: 1.0 for the raw-d
  bias row), range-reduced with the fp32 magic-number trick, evaluated by
  ScalarE Sin, then a K=21 matmul against [W_rbf; b_rbf] gives w*d in PSUM;
  scaling by env/d and the gathered phi finishes the edge.
- Output is written fp16 (halves both the donated-zero upload and the
  fetch) and upcast on host; worst-case per-element error ~0.3%.
"""
import sys

sys.path.insert(0, "/opt/trn_rl_repo")

import numpy as np

try:  # cache XLA executables across calls (the runner re-jits every call)
    import jax
    jax.config.update("jax_compilation_cache_dir", "/tmp/jax_comp_cache")
    jax.config.update("jax_persistent_cache_min_entry_size_bytes", 0)
    jax.config.update("jax_persistent_cache_min_compile_time_secs", 0)
except Exception:
    pass

import concourse.tile as tile
from concourse import bass, bacc, mybir
from concourse.bass_utils import run_bass_kernel_spmd

F32 = mybir.dt.float32
F16 = mybir.dt.float16
I32 = mybir.dt.int32

N_CORES = 8
N_ATOMS = 50000
N_EDGES = 800000
D = 128
NB = 20
CUTOFF = 5.0
MAGIC = float(np.float32(1.5 * 2**23))

NPC = N_ATOMS // N_CORES          # 6250 nodes per core
NPT = 6656                        # table rows = 13*512 (>= NPC)
NCH_NODE = NPT // 512             # 13 node MLP chunks
GCH = 3072                        # edges per gather/output chunk


def build_nc(n_gchunks):
    nc = bacc.Bacc(None, target_bir_lowering=False)
    E_pad = n_gchunks * GCH

    # constants packed into two tensors to minimize per-transfer latency:
    # pk32 [128, 770]: 0:128 wext | 128:256 ident | 256:768 cmat (rows 0:12)
    #                  | 768 b1 | 769 -pi/2
    # pk16 [128, 512]: 0:128 W1 | 128:256 W2 | row0 of 256:384 ones
    #                  | row0 of 384:512 b2
    s_jT = nc.dram_tensor("s_jT", [D, NPT], F16, kind="ExternalInput")
    pk32 = nc.dram_tensor("pk32", [D, 770], F32, kind="ExternalInput")
    pk16 = nc.dram_tensor("pk16", [D, 512], F16, kind="ExternalInput")
    idx32 = nc.dram_tensor("idx32", [E_pad, 1], I32, kind="ExternalInput")
    dist3 = nc.dram_tensor("dist3", [E_pad // D, D], F32, kind="ExternalInput")
    out_dev = nc.dram_tensor("out_dev", [E_pad, D], F16, kind="ExternalOutput")

    inv = nc.dram_tensor("inv", [NPT, D], F32, kind="Internal")

    with tile.TileContext(nc) as tc:
        with tc.tile_pool(name="const", bufs=1) as cpool, \
             tc.tile_pool(name="mlp", bufs=3) as mpool, \
             tc.tile_pool(name="bigp", bufs=2, space="PSUM") as bigp, \
             tc.tile_pool(name="smallp", bufs=4, space="PSUM") as smallp, \
             tc.tile_pool(name="dtpp", bufs=2, space="PSUM") as dtpp, \
             tc.tile_pool(name="edge", bufs=3) as epool, \
             tc.tile_pool(name="big", bufs=2) as bpool:

            pc32 = cpool.tile([D, 770], F32)
            nc.sync.dma_start(out=pc32[:], in_=pk32[:])
            pc16 = cpool.tile([D, 512], F16)
            nc.sync.dma_start(out=pc16[:], in_=pk16[:])
            wext_sb = pc32[:, 0:D]
            id_sb = pc32[:, D:2 * D]
            cm_sb = pc32[0:12, 2 * D:2 * D + 512]
            b1_sb = pc32[:, 768:769]
            nhp_sb = pc32[:, 769:770]
            w1_sb = pc16[:, 0:D]
            w2_sb = pc16[:, D:2 * D]
            ones_sb = pc16[0:1, 2 * D:3 * D]
            b2r_sb = pc16[0:1, 3 * D:4 * D]

            # ---- Phase 1: node MLP -> inv table (node-major rows) ----
            for i in range(NCH_NODE):
                s_t = mpool.tile([D, 512], F16, tag="s")
                nc.sync.dma_start(out=s_t[:], in_=s_jT[:, i * 512:(i + 1) * 512])
                ph = bigp.tile([D, 512], F32, tag="mm512")
                nc.tensor.matmul(out=ph[:], lhsT=w1_sb[:], rhs=s_t[:],
                                 start=True, stop=True)
                h_t = mpool.tile([D, 512], F16, tag="h")
                nc.scalar.activation(out=h_t[:], in_=ph[:],
                                     func=mybir.ActivationFunctionType.Silu,
                                     bias=b1_sb[:, 0:1], scale=1.0)
                for b in range(4):
                    pt = smallp.tile([D, D], F32, tag="mm128")
                    nc.tensor.matmul(out=pt[:],
                                     lhsT=h_t[:, b * D:(b + 1) * D],
                                     rhs=w2_sb[:], start=True, stop=False)
                    nc.tensor.matmul(out=pt[:], lhsT=ones_sb[:],
                                     rhs=b2r_sb[:], start=False, stop=True)
                    ot = mpool.tile([D, D], F32, tag="ot")
                    nc.scalar.copy(out=ot[:], in_=pt[:])
                    n0 = i * 512 + b * D
                    nc.sync.dma_start(out=inv[n0:n0 + D, :], in_=ot[:])

            # ---- Phase 2: edges ----
            for g in range(n_gchunks):
                ix = epool.tile([D, GCH // D], I32, tag="ix")
                nc.sync.dma_start(
                    out=ix[:],
                    in_=idx32[g * GCH:(g + 1) * GCH, :].rearrange(
                        "(s p) o -> p (s o)", p=D))
                phi = bpool.tile([D, GCH // D, D], F32, tag="phi")
                for s in range(GCH // D):
                    nc.gpsimd.indirect_dma_start(
                        out=phi[:, s, :], out_offset=None, in_=inv[:],
                        in_offset=bass.IndirectOffsetOnAxis(
                            ap=ix[:, s:s + 1], axis=0))
                out_sb = bpool.tile([D, GCH // D, D], F16, tag="osb")
                for c2 in range(GCH // 1536):
                    blk = g * 2 + c2
                    # dist rows for this 1536-edge block: [12, 128]
                    dt3 = epool.tile([12, D], F32, tag="dt3")
                    nc.sync.dma_start(out=dt3[:],
                                      in_=dist3[blk * 12:(blk + 1) * 12, :])
                    # transpose -> [128, 12] per-partition dist scalars
                    dtp = dtpp.tile([D, 12], F32, tag="dtp")
                    nc.tensor.transpose(out=dtp[:], in_=dt3[:],
                                        identity=id_sb[0:12, 0:12])
                    dt = epool.tile([D, 12], F32, tag="dt")
                    nc.scalar.copy(out=dt[:], in_=dtp[:])
                    rd = epool.tile([D, 12], F32, tag="rd")
                    nc.vector.reciprocal(out=rd[:], in_=dt[:])
                    cs = epool.tile([D, 12], F32, tag="cs")
                    nc.scalar.activation(out=cs[:], in_=dt[:],
                                         func=mybir.ActivationFunctionType.Sin,
                                         scale=float(np.pi / CUTOFF),
                                         bias=nhp_sb[:, 0:1])
                    env = epool.tile([D, 12], F32, tag="env")
                    nc.vector.tensor_scalar(out=env[:], in0=cs[:],
                                            scalar1=-0.5, scalar2=0.5,
                                            op0=mybir.AluOpType.mult,
                                            op1=mybir.AluOpType.add)
                    scl = epool.tile([D, 12], F32, tag="scl")
                    nc.vector.tensor_tensor(out=scl[:], in0=env[:], in1=rd[:],
                                            op=mybir.AluOpType.mult)
                    # u[32j+n, 128k+e] = coef_n * d[(3k+j)*128+e] via K=3 matmuls
                    u = bigp.tile([D, 512], F32, tag="mm512")
                    for k in range(4):
                        nc.tensor.matmul(out=u[:, k * D:(k + 1) * D],
                                         lhsT=cm_sb[:, k * D:(k + 1) * D],
                                         rhs=dt3[:],
                                         start=True, stop=True)
                    kf = epool.tile([D, 512], F32, tag="kf")
                    nc.vector.tensor_scalar(out=kf[:], in0=u[:],
                                            scalar1=MAGIC, scalar2=MAGIC,
                                            op0=mybir.AluOpType.add,
                                            op1=mybir.AluOpType.subtract)
                    v = epool.tile([D, 512], F32, tag="v")
                    nc.vector.tensor_tensor(out=v[:], in0=u[:], in1=kf[:],
                                            op=mybir.AluOpType.subtract)
                    db = epool.tile([D, 512], F32, tag="db")
                    # full-tile copy seeds the raw-d rows (32j+20); Sin then
                    # overwrites rows 32j..32j+19 (ops must be 32-part aligned)
                    nc.vector.tensor_copy(out=db[:], in_=u[:])
                    for j in range(3):
                        nc.scalar.activation(
                            out=db[32 * j:32 * j + NB, :],
                            in_=v[32 * j:32 * j + NB, :],
                            func=mybir.ActivationFunctionType.Sin,
                            scale=float(2 * np.pi))
                    for t in range(12):
                        k, j = t // 3, t % 3
                        pw = smallp.tile([D, D], F32, tag="mm128")
                        nc.tensor.matmul(
                            out=pw[:],
                            lhsT=db[32 * j:32 * j + NB + 1, k * D:(k + 1) * D],
                            rhs=wext_sb[32 * j:32 * j + NB + 1, :],
                            start=True, stop=True)
                        ws = epool.tile([D, D], F32, tag="ws")
                        nc.scalar.activation(
                            out=ws[:], in_=pw[:],
                            func=mybir.ActivationFunctionType.Copy,
                            scale=scl[:, t:t + 1])
                        slot = c2 * 12 + t
                        nc.vector.tensor_tensor(
                            out=out_sb[:, slot, :], in0=ws[:],
                            in1=phi[:, slot, :], op=mybir.AluOpType.mult)
                nc.sync.dma_start(
                    out=out_dev[g * GCH:(g + 1) * GCH, :].rearrange(
                        "(s p) f -> p s f", p=D),
                    in_=out_sb[:])
    nc.finalize()
    return nc


_NC_CACHE = {}


def kernel(s_j, dist, nbrs, W1, b1, W2, b2, W_rbf, b_rbf):
    s_j = np.asarray(s_j, dtype=np.float32)
    dist = np.asarray(dist, dtype=np.float32)
    j_idx = np.asarray(nbrs)[:, 1].astype(np.int32)

    core = j_idx // NPC
    order = np.argsort(core, kind="stable")
    counts = np.bincount(core, minlength=N_CORES)
    n_g = int((counts.max() + GCH - 1) // GCH)
    E_pad = n_g * GCH

    w21 = np.concatenate([np.asarray(W_rbf, np.float32),
                          np.asarray(b_rbf, np.float32)[None, :]], axis=0)
    wext = np.zeros((D, D), dtype=np.float32)
    for qj in range(3):
        wext[32 * qj:32 * qj + NB + 1] = w21
    # cmat[t, 128k+32j+n] = delta(t, 3k+j) * coef_n
    # coef_n = (n+1)/10 (n<20), 1.0 (n=20, the raw-d row), 0 else
    coef = np.zeros(32, dtype=np.float32)
    coef[:NB] = (np.arange(NB) + 1) / 10.0
    coef[NB] = 1.0
    cmat = np.zeros((12, 512), dtype=np.float32)
    for k in range(4):
        for j in range(3):
            cmat[3 * k + j, 128 * k + 32 * j:128 * k + 32 * j + 32] = coef
    pk32 = np.zeros((D, 770), dtype=np.float32)
    pk32[:, 0:D] = wext
    pk32[:, D:2 * D] = np.eye(D, dtype=np.float32)
    pk32[0:12, 2 * D:2 * D + 512] = cmat
    pk32[:, 768] = np.asarray(b1, np.float32)
    pk32[:, 769] = -np.pi / 2
    pk16 = np.zeros((D, 512), dtype=np.float16)
    pk16[:, 0:D] = np.asarray(W1, np.float32).astype(np.float16)
    pk16[:, D:2 * D] = np.asarray(W2, np.float32).astype(np.float16)
    pk16[0, 2 * D:3 * D] = 1.0
    pk16[0, 3 * D:4 * D] = np.asarray(b2, np.float32).astype(np.float16)
    common = {"pk32": pk32, "pk16": pk16}

    starts = np.zeros(N_CORES + 1, dtype=np.int64)
    starts[1:] = np.cumsum(counts)
    in_maps = []
    for c in range(N_CORES):
        sel = order[starts[c]:starts[c + 1]]
        n_c = counts[c]
        idx_pad = np.zeros(E_pad, dtype=np.int32)
        dist_pad = np.ones(E_pad, dtype=np.float32)
        idx_pad[:n_c] = j_idx[sel] - c * NPC
        dist_pad[:n_c] = dist[sel]
        s_T = np.zeros((D, NPT), dtype=np.float16)
        s_T[:, :NPC] = s_j[c * NPC:(c + 1) * NPC].T
        in_maps.append(dict(common, s_jT=s_T,
                            idx32=idx_pad.reshape(-1, 1),
                            dist3=dist_pad.reshape(-1, D)))

    if n_g not in _NC_CACHE:
        _NC_CACHE[n_g] = build_nc(n_g)
    nc = _NC_CACHE[n_g]

    res = run_bass_kernel_spmd(nc, in_maps, list(range(N_CORES)))
    valid = np.concatenate(
        [res.results[c]["out_dev"][:counts[c]] for c in range(N_CORES)], axis=0)
    out = np.empty((N_EDGES, D), dtype=np.float32)
    out[order] = valid.astype(np.float32)
    return out


# revision 31
# speedup vs baseline: 1.0859x; 1.0859x over previous
"""Trainium2 Bass kernel for InvariantMessage GNN message passing.

out[e, :] = (MLP(s_j)[nbrs[e,1]]) * ((rbf(dist[e]) @ W_rbf + b_rbf) * env(dist[e]))

The axon tunnel (~30-90 MB/s) dominates wall time, so the design minimizes
host<->device bytes; on-device compute is ~0.1 s.

Strategy (8 cores, node-sharded):
- Nodes are split 6250/core; each EDGE is assigned to the core that owns its
  gathered node nbrs[e,1], so every gather is core-local (no collectives) and
  s_j is uploaded exactly once across the fleet (fp16, 1.7 MB/core).
- Each core runs the MLP on its 6250 nodes into an Internal-DRAM table
  (no zero upload), then per 3072-edge chunk gathers phi rows with
  indirect_dma_start (128 rows/instruction -- the HW-validated semantics).
- dist is uploaded once in a [12,128]-per-1536-edge layout; the quadrant-
  packed sin-argument tile is built on device by K=3 outer-product matmuls
  against a coefficient matrix (row n<20: (n+1)/10, row 20: 1.0 for the raw-d
  bias row), range-reduced with the fp32 magic-number trick, evaluated by
  ScalarE Sin, then a K=21 matmul against [W_rbf; b_rbf] gives w*d in PSUM;
  scaling by env/d and the gathered phi finishes the edge.
- Output is written fp16 (halves both the donated-zero upload and the
  fetch) and upcast on host; worst-case per-element error ~0.3%.
"""
import sys

sys.path.insert(0, "/opt/trn_rl_repo")

import numpy as np

try:  # cache XLA executables across calls (the runner re-jits every call)
    import jax
    jax.config.update("jax_compilation_cache_dir", "/tmp/jax_comp_cache")
    jax.config.update("jax_persistent_cache_min_entry_size_bytes", 0)
    jax.config.update("jax_persistent_cache_min_compile_time_secs", 0)
except Exception:
    pass

import concourse.tile as tile
from concourse import bass, bacc, mybir
from concourse.bass_utils import run_bass_kernel_spmd

F32 = mybir.dt.float32
F16 = mybir.dt.float16
I32 = mybir.dt.int32

N_CORES = 8
N_ATOMS = 50000
N_EDGES = 800000
D = 128
NB = 20
CUTOFF = 5.0
MAGIC = float(np.float32(1.5 * 2**23))

NPC = N_ATOMS // N_CORES          # 6250 nodes per core
NPT = 6656                        # table rows = 13*512 (>= NPC)
NCH_NODE = NPT // 512             # 13 node MLP chunks
GCH = 3072                        # edges per gather/output chunk


def build_nc(n_gchunks):
    nc = bacc.Bacc(None, target_bir_lowering=False)
    E_pad = n_gchunks * GCH
    NR_E = E_pad // D

    # All inputs ride in ONE int32 blob (the axon tunnel charges ~20 ms
    # latency per array shard, so fewer arrays = faster upload):
    #   rows [0, NR_E)         idx32, row r = idx[128r : 128r+128]
    #   rows [NR_E, 2 NR_E)    dist fp32 bits, same row structure
    #   rows [S0, S0+3328)     s_jT fp16 [128, 6656]; row S0+26p+k holds
    #                          int32 cols 128k:128k+128 of partition p
    #   rows [P0, P0+896)      pk32 fp32 [128, 896]: 0:128 wext | 128:256
    #                          ident | 256:768 cmat(rows 0:12) | 768 b1
    #                          | 769 -pi/2 (row p*7+k layout)
    #   rows [Q0, Q0+256)      pk16 fp16 [128, 512]: 0:128 W1 | 128:256 W2
    #                          | row0 256:384 ones | row0 384:512 b2
    S0 = 2 * NR_E
    P0 = S0 + 3328
    Q0 = P0 + 896
    R_total = Q0 + 256
    blob = nc.dram_tensor("blob", [R_total, D], I32, kind="ExternalInput")
    out_dev = nc.dram_tensor("out_dev", [E_pad, D], F16, kind="ExternalOutput")

    inv = nc.dram_tensor("inv", [NPT, D], F32, kind="Internal")

    s_ap = blob[S0:S0 + 3328, :].rearrange("(p k) e -> p (k e)", p=D)
    pk32_ap = blob[P0:P0 + 896, :].rearrange(
        "(p k) e -> p (k e)", p=D).bitcast(F32)
    pk16_ap = blob[Q0:Q0 + 256, :].rearrange(
        "(p k) e -> p (k e)", p=D).bitcast(F16)

    with tile.TileContext(nc) as tc:
        with tc.tile_pool(name="const", bufs=1) as cpool, \
             tc.tile_pool(name="mlp", bufs=3) as mpool, \
             tc.tile_pool(name="bigp", bufs=2, space="PSUM") as bigp, \
             tc.tile_pool(name="smallp", bufs=4, space="PSUM") as smallp, \
             tc.tile_pool(name="dtpp", bufs=2, space="PSUM") as dtpp, \
             tc.tile_pool(name="edge", bufs=3) as epool, \
             tc.tile_pool(name="big", bufs=2) as bpool:

            pc32 = cpool.tile([D, 896], F32)
            nc.sync.dma_start(out=pc32[:], in_=pk32_ap)
            pc16 = cpool.tile([D, 512], F16)
            nc.sync.dma_start(out=pc16[:], in_=pk16_ap)
            wext_sb = pc32[:, 0:D]
            id_sb = pc32[:, D:2 * D]
            cm_sb = pc32[0:12, 2 * D:2 * D + 512]
            b1_sb = pc32[:, 768:769]
            nhp_sb = pc32[:, 769:770]
            w1_sb = pc16[:, 0:D]
            w2_sb = pc16[:, D:2 * D]
            ones_sb = pc16[0:1, 2 * D:3 * D]
            b2r_sb = pc16[0:1, 3 * D:4 * D]

            # ---- Phase 1: node MLP -> inv table (node-major rows) ----
            for i in range(NCH_NODE):
                s_t = mpool.tile([D, 512], F16, tag="s")
                nc.sync.dma_start(
                    out=s_t[:],
                    in_=s_ap[:, i * 256:(i + 1) * 256].bitcast(F16))
                ph = bigp.tile([D, 512], F32, tag="mm512")
                nc.tensor.matmul(out=ph[:], lhsT=w1_sb[:], rhs=s_t[:],
                                 start=True, stop=True)
                h_t = mpool.tile([D, 512], F16, tag="h")
                nc.scalar.activation(out=h_t[:], in_=ph[:],
                                     func=mybir.ActivationFunctionType.Silu,
                                     bias=b1_sb[:, 0:1], scale=1.0)
                for b in range(4):
                    pt = smallp.tile([D, D], F32, tag="mm128")
                    nc.tensor.matmul(out=pt[:],
                                     lhsT=h_t[:, b * D:(b + 1) * D],
                                     rhs=w2_sb[:], start=True, stop=False)
                    nc.tensor.matmul(out=pt[:], lhsT=ones_sb[:],
                                     rhs=b2r_sb[:], start=False, stop=True)
                    ot = mpool.tile([D, D], F32, tag="ot")
                    nc.scalar.copy(out=ot[:], in_=pt[:])
                    n0 = i * 512 + b * D
                    nc.sync.dma_start(out=inv[n0:n0 + D, :], in_=ot[:])

            # ---- Phase 2: edges ----
            for g in range(n_gchunks):
                ix = epool.tile([D, GCH // D], I32, tag="ix")
                nc.sync.dma_start(
                    out=ix[:],
                    in_=blob[g * (GCH // D):(g + 1) * (GCH // D), :].rearrange(
                        "s p -> p s"))
                phi = bpool.tile([D, GCH // D, D], F32, tag="phi")
                for s in range(GCH // D):
                    nc.gpsimd.indirect_dma_start(
                        out=phi[:, s, :], out_offset=None, in_=inv[:],
                        in_offset=bass.IndirectOffsetOnAxis(
                            ap=ix[:, s:s + 1], axis=0))
                out_sb = bpool.tile([D, GCH // D, D], F16, tag="osb")
                for c2 in range(GCH // 1536):
                    blk = g * 2 + c2
                    # dist rows for this 1536-edge block: [12, 128]
                    dt3 = epool.tile([12, D], F32, tag="dt3")
                    nc.sync.dma_start(
                        out=dt3[:],
                        in_=blob[NR_E + blk * 12:NR_E + (blk + 1) * 12,
                                 :].bitcast(F32))
                    # transpose -> [128, 12] per-partition dist scalars
                    dtp = dtpp.tile([D, 12], F32, tag="dtp")
                    nc.tensor.transpose(out=dtp[:], in_=dt3[:],
                                        identity=id_sb[0:12, 0:12])
                    dt = epool.tile([D, 12], F32, tag="dt")
                    nc.scalar.copy(out=dt[:], in_=dtp[:])
                    rd = epool.tile([D, 12], F32, tag="rd")
                    nc.vector.reciprocal(out=rd[:], in_=dt[:])
                    cs = epool.tile([D, 12], F32, tag="cs")
                    nc.scalar.activation(out=cs[:], in_=dt[:],
                                         func=mybir.ActivationFunctionType.Sin,
                                         scale=float(np.pi / CUTOFF),
                                         bias=nhp_sb[:, 0:1])
                    env = epool.tile([D, 12], F32, tag="env")
                    nc.vector.tensor_scalar(out=env[:], in0=cs[:],
                                            scalar1=-0.5, scalar2=0.5,
                                            op0=mybir.AluOpType.mult,
                                            op1=mybir.AluOpType.add)
                    scl = epool.tile([D, 12], F32, tag="scl")
                    nc.vector.tensor_tensor(out=scl[:], in0=env[:], in1=rd[:],
                                            op=mybir.AluOpType.mult)
                    # u[32j+n, 128k+e] = coef_n * d[(3k+j)*128+e] via K=3 matmuls
                    u = bigp.tile([D, 512], F32, tag="mm512")
                    for k in range(4):
                        nc.tensor.matmul(out=u[:, k * D:(k + 1) * D],
                                         lhsT=cm_sb[:, k * D:(k + 1) * D],
                                         rhs=dt3[:],
                                         start=True, stop=True)
                    kf = epool.tile([D, 512], F32, tag="kf")
                    nc.vector.tensor_scalar(out=kf[:], in0=u[:],
                                            scalar1=MAGIC, scalar2=MAGIC,
                                            op0=mybir.AluOpType.add,
                                            op1=mybir.AluOpType.subtract)
                    v = epool.tile([D, 512], F32, tag="v")
                    nc.vector.tensor_tensor(out=v[:], in0=u[:], in1=kf[:],
                                            op=mybir.AluOpType.subtract)
                    db = epool.tile([D, 512], F32, tag="db")
                    # full-tile copy seeds the raw-d rows (32j+20); Sin then
                    # overwrites rows 32j..32j+19 (ops must be 32-part aligned)
                    nc.vector.tensor_copy(out=db[:], in_=u[:])
                    for j in range(3):
                        nc.scalar.activation(
                            out=db[32 * j:32 * j + NB, :],
                            in_=v[32 * j:32 * j + NB, :],
                            func=mybir.ActivationFunctionType.Sin,
                            scale=float(2 * np.pi))
                    for t in range(12):
                        k, j = t // 3, t % 3
                        pw = smallp.tile([D, D], F32, tag="mm128")
                        nc.tensor.matmul(
                            out=pw[:],
                            lhsT=db[32 * j:32 * j + NB + 1, k * D:(k + 1) * D],
                            rhs=wext_sb[32 * j:32 * j + NB + 1, :],
                            start=True, stop=True)
                        ws = epool.tile([D, D], F32, tag="ws")
                        nc.scalar.activation(
                            out=ws[:], in_=pw[:],
                            func=mybir.ActivationFunctionType.Copy,
                            scale=scl[:, t:t + 1])
                        slot = c2 * 12 + t
                        nc.vector.tensor_tensor(
                            out=out_sb[:, slot, :], in0=ws[:],
                            in1=phi[:, slot, :], op=mybir.AluOpType.mult)
                nc.sync.dma_start(
                    out=out_dev[g * GCH:(g + 1) * GCH, :].rearrange(
                        "(s p) f -> p s f", p=D),
                    in_=out_sb[:])
    nc.finalize()
    return nc


_NC_CACHE = {}


def kernel(s_j, dist, nbrs, W1, b1, W2, b2, W_rbf, b_rbf):
    s_j = np.asarray(s_j, dtype=np.float32)
    dist = np.asarray(dist, dtype=np.float32)
    j_idx = np.asarray(nbrs)[:, 1].astype(np.int32)

    core = j_idx // NPC
    order = np.argsort(core, kind="stable")
    counts = np.bincount(core, minlength=N_CORES)
    n_g = int((counts.max() + GCH - 1) // GCH)
    E_pad = n_g * GCH

    w21 = np.concatenate([np.asarray(W_rbf, np.float32),
                          np.asarray(b_rbf, np.float32)[None, :]], axis=0)
    wext = np.zeros((D, D), dtype=np.float32)
    for qj in range(3):
        wext[32 * qj:32 * qj + NB + 1] = w21
    # cmat[t, 128k+32j+n] = delta(t, 3k+j) * coef_n
    # coef_n = (n+1)/10 (n<20), 1.0 (n=20, the raw-d row), 0 else
    coef = np.zeros(32, dtype=np.float32)
    coef[:NB] = (np.arange(NB) + 1) / 10.0
    coef[NB] = 1.0
    cmat = np.zeros((12, 512), dtype=np.float32)
    for k in range(4):
        for j in range(3):
            cmat[3 * k + j, 128 * k + 32 * j:128 * k + 32 * j + 32] = coef
    pk32 = np.zeros((D, 896), dtype=np.float32)
    pk32[:, 0:D] = wext
    pk32[:, D:2 * D] = np.eye(D, dtype=np.float32)
    pk32[0:12, 2 * D:2 * D + 512] = cmat
    pk32[:, 768] = np.asarray(b1, np.float32)
    pk32[:, 769] = -np.pi / 2
    pk16 = np.zeros((D, 512), dtype=np.float16)
    pk16[:, 0:D] = np.asarray(W1, np.float32).astype(np.float16)
    pk16[:, D:2 * D] = np.asarray(W2, np.float32).astype(np.float16)
    pk16[0, 2 * D:3 * D] = 1.0
    pk16[0, 3 * D:4 * D] = np.asarray(b2, np.float32).astype(np.float16)
    pk32_rows = pk32.view(np.int32).reshape(-1, D)
    pk16_rows = pk16.view(np.int32).reshape(-1, D)

    NR_E = E_pad // D
    S0 = 2 * NR_E
    P0 = S0 + 3328
    Q0 = P0 + 896
    R_total = Q0 + 256

    starts = np.zeros(N_CORES + 1, dtype=np.int64)
    starts[1:] = np.cumsum(counts)
    in_maps = []
    for c in range(N_CORES):
        sel = order[starts[c]:starts[c + 1]]
        n_c = counts[c]
        blob = np.zeros((R_total, D), dtype=np.int32)
        idx_pad = blob[0:NR_E].reshape(-1)
        idx_pad[:n_c] = j_idx[sel] - c * NPC
        dist_pad = blob[NR_E:S0].reshape(-1).view(np.float32)
        dist_pad[:] = 1.0
        dist_pad[:n_c] = dist[sel]
        s_T = np.zeros((D, NPT), dtype=np.float16)
        s_T[:, :NPC] = s_j[c * NPC:(c + 1) * NPC].T
        blob[S0:P0] = s_T.view(np.int32).reshape(-1, D)
        blob[P0:Q0] = pk32_rows
        blob[Q0:R_total] = pk16_rows
        in_maps.append({"blob": blob})

    if n_g not in _NC_CACHE:
        _NC_CACHE[n_g] = build_nc(n_g)
    nc = _NC_CACHE[n_g]

    res = run_bass_kernel_spmd(nc, in_maps, list(range(N_CORES)))
    valid = np.concatenate(
        [res.results[c]["out_dev"][:counts[c]] for c in range(N_CORES)], axis=0)
    out = np.empty((N_EDGES, D), dtype=np.float32)
    out[order] = valid.astype(np.float32)
    return out


# revision 32
# speedup vs baseline: 1.1454x; 1.0548x over previous
"""Trainium2 Bass kernel for InvariantMessage GNN message passing.

out[e, :] = (MLP(s_j)[nbrs[e,1]]) * ((rbf(dist[e]) @ W_rbf + b_rbf) * env(dist[e]))

The axon tunnel (~40 MB/s D2H uncompressed, ~100 MB/s H2D on compressible
data, ~20 ms latency per array shard) dominates wall time -- on-device
compute is ~0.1 s -- so the design minimizes host<->device bytes and
transfer count:
- Nodes are split 6250/core; each EDGE is assigned to the core that owns its
  gathered node nbrs[e,1] (host knows the indices), so every gather is
  core-local (no collectives) and s_j is uploaded exactly once across the
  fleet (fp16, 1.7 MB/core) instead of replicated x8 in fp32.
- ALL per-core inputs (idx, dist, s_j^T fp16, packed fp32/fp16 constants)
  ride in ONE int32 blob (~3.1 MB/core); device-side slices use
  rearrange+bitcast. Fewer arrays = fewer per-shard latency hits.
- Each core runs the MLP (fp16 weights/activations, fp32 PSUM; bias b2 is
  added by an accumulating ones-outer-product matmul, giving [node, feat]
  orientation without a transpose) on its 6250 nodes into an Internal-DRAM
  table -- Internal kind avoids uploading a zero table. Then per 3072-edge
  chunk phi rows are fetched with indirect_dma_start (128 rows/instruction,
  the HW-validated semantics; int16 dma_gather hung the device).
- dist is uploaded once ([12,128] rows per 1536-edge block). The quadrant-
  packed sin-argument tile u[32j+n, 128k+e] = coef_n * d is built on device
  by K=12 matmuls against a selection-coefficient matrix (coef_n=(n+1)/10
  for n<20, 1.0 for n=20, the raw-d row the K=21 rbf matmul uses for its
  b_rbf term), range-reduced to [-.5,.5] with the fp32 magic-number trick,
  evaluated by ScalarE Sin (rows seeded by a full-tile DVE copy since engine
  ops must be 32-partition aligned), then a K=21 matmul against
  [W_rbf; b_rbf] gives w*d in PSUM; scaling by env/d (from a tensor-engine
  transpose of the dist rows) and the gathered phi finishes the edge.
- Output is fp16 (halves both the donated-zero upload run_bass_via_pjrt
  forces and the uncompressible fetch) and upcast on host; worst-case
  per-element error ~0.3%, observed 6e-4 global-relative.
Warm-call budget: ~5.1 s output fetch + ~2.0 s donated-zero upload +
~0.6 s blob upload + ~0.45 s executable load (jax persistent cache) +
~0.12 s execute = ~8.2 s, vs 28.3 s for the previous kernel.
"""
import sys

sys.path.insert(0, "/opt/trn_rl_repo")

import numpy as np

try:  # cache XLA executables across calls (the runner re-jits every call)
    import jax
    jax.config.update("jax_compilation_cache_dir", "/tmp/jax_comp_cache")
    jax.config.update("jax_persistent_cache_min_entry_size_bytes", 0)
    jax.config.update("jax_persistent_cache_min_compile_time_secs", 0)
except Exception:
    pass

import concourse.tile as tile
from concourse import bass, bacc, mybir
from concourse.bass_utils import run_bass_kernel_spmd

F32 = mybir.dt.float32
F16 = mybir.dt.float16
I32 = mybir.dt.int32

N_CORES = 8
N_ATOMS = 50000
N_EDGES = 800000
D = 128
NB = 20
CUTOFF = 5.0
MAGIC = float(np.float32(1.5 * 2**23))

NPC = N_ATOMS // N_CORES          # 6250 nodes per core
NPT = 6656                        # table rows = 13*512 (>= NPC)
NCH_NODE = NPT // 512             # 13 node MLP chunks
GCH = 3072                        # edges per gather/output chunk


def build_nc(n_gchunks):
    nc = bacc.Bacc(None, target_bir_lowering=False)
    E_pad = n_gchunks * GCH
    NR_E = E_pad // D

    # All inputs ride in ONE int32 blob (the axon tunnel charges ~20 ms
    # latency per array shard, so fewer arrays = faster upload):
    #   rows [0, NR_E)         idx32, row r = idx[128r : 128r+128]
    #   rows [NR_E, 2 NR_E)    dist fp32 bits, same row structure
    #   rows [S0, S0+3328)     s_jT fp16 [128, 6656]; row S0+26p+k holds
    #                          int32 cols 128k:128k+128 of partition p
    #   rows [P0, P0+896)      pk32 fp32 [128, 896]: 0:128 wext | 128:256
    #                          ident | 256:768 cmat(rows 0:12) | 768 b1
    #                          | 769 -pi/2 (row p*7+k layout)
    #   rows [Q0, Q0+256)      pk16 fp16 [128, 512]: 0:128 W1 | 128:256 W2
    #                          | row0 256:384 ones | row0 384:512 b2
    S0 = 2 * NR_E
    P0 = S0 + 3328
    Q0 = P0 + 896
    R_total = Q0 + 256
    blob = nc.dram_tensor("blob", [R_total, D], I32, kind="ExternalInput")
    out_dev = nc.dram_tensor("out_dev", [E_pad, D], F16, kind="ExternalOutput")

    inv = nc.dram_tensor("inv", [NPT, D], F32, kind="Internal")

    s_ap = blob[S0:S0 + 3328, :].rearrange("(p k) e -> p (k e)", p=D)
    pk32_ap = blob[P0:P0 + 896, :].rearrange(
        "(p k) e -> p (k e)", p=D).bitcast(F32)
    pk16_ap = blob[Q0:Q0 + 256, :].rearrange(
        "(p k) e -> p (k e)", p=D).bitcast(F16)

    with tile.TileContext(nc) as tc:
        with tc.tile_pool(name="const", bufs=1) as cpool, \
             tc.tile_pool(name="mlp", bufs=3) as mpool, \
             tc.tile_pool(name="bigp", bufs=2, space="PSUM") as bigp, \
             tc.tile_pool(name="smallp", bufs=4, space="PSUM") as smallp, \
             tc.tile_pool(name="dtpp", bufs=2, space="PSUM") as dtpp, \
             tc.tile_pool(name="edge", bufs=3) as epool, \
             tc.tile_pool(name="big", bufs=2) as bpool:

            pc32 = cpool.tile([D, 896], F32)
            nc.sync.dma_start(out=pc32[:], in_=pk32_ap)
            pc16 = cpool.tile([D, 512], F16)
            nc.sync.dma_start(out=pc16[:], in_=pk16_ap)
            wext_sb = pc32[:, 0:D]
            id_sb = pc32[:, D:2 * D]
            cm_sb = pc32[0:12, 2 * D:2 * D + 512]
            b1_sb = pc32[:, 768:769]
            nhp_sb = pc32[:, 769:770]
            w1_sb = pc16[:, 0:D]
            w2_sb = pc16[:, D:2 * D]
            ones_sb = pc16[0:1, 2 * D:3 * D]
            b2r_sb = pc16[0:1, 3 * D:4 * D]

            # ---- Phase 1: node MLP -> inv table (node-major rows) ----
            for i in range(NCH_NODE):
                s_t = mpool.tile([D, 512], F16, tag="s")
                nc.sync.dma_start(
                    out=s_t[:],
                    in_=s_ap[:, i * 256:(i + 1) * 256].bitcast(F16))
                ph = bigp.tile([D, 512], F32, tag="mm512")
                nc.tensor.matmul(out=ph[:], lhsT=w1_sb[:], rhs=s_t[:],
                                 start=True, stop=True)
                h_t = mpool.tile([D, 512], F16, tag="h")
                nc.scalar.activation(out=h_t[:], in_=ph[:],
                                     func=mybir.ActivationFunctionType.Silu,
                                     bias=b1_sb[:, 0:1], scale=1.0)
                for b in range(4):
                    pt = smallp.tile([D, D], F32, tag="mm128")
                    nc.tensor.matmul(out=pt[:],
                                     lhsT=h_t[:, b * D:(b + 1) * D],
                                     rhs=w2_sb[:], start=True, stop=False)
                    nc.tensor.matmul(out=pt[:], lhsT=ones_sb[:],
                                     rhs=b2r_sb[:], start=False, stop=True)
                    ot = mpool.tile([D, D], F32, tag="ot")
                    nc.scalar.copy(out=ot[:], in_=pt[:])
                    n0 = i * 512 + b * D
                    nc.sync.dma_start(out=inv[n0:n0 + D, :], in_=ot[:])

            # ---- Phase 2: edges ----
            for g in range(n_gchunks):
                ix = epool.tile([D, GCH // D], I32, tag="ix")
                nc.sync.dma_start(
                    out=ix[:],
                    in_=blob[g * (GCH // D):(g + 1) * (GCH // D), :].rearrange(
                        "s p -> p s"))
                phi = bpool.tile([D, GCH // D, D], F32, tag="phi")
                for s in range(GCH // D):
                    nc.gpsimd.indirect_dma_start(
                        out=phi[:, s, :], out_offset=None, in_=inv[:],
                        in_offset=bass.IndirectOffsetOnAxis(
                            ap=ix[:, s:s + 1], axis=0))
                out_sb = bpool.tile([D, GCH // D, D], F16, tag="osb")
                for c2 in range(GCH // 1536):
                    blk = g * 2 + c2
                    # dist rows for this 1536-edge block: [12, 128]
                    dt3 = epool.tile([12, D], F32, tag="dt3")
                    nc.sync.dma_start(
                        out=dt3[:],
                        in_=blob[NR_E + blk * 12:NR_E + (blk + 1) * 12,
                                 :].bitcast(F32))
                    # transpose -> [128, 12] per-partition dist scalars
                    dtp = dtpp.tile([D, 12], F32, tag="dtp")
                    nc.tensor.transpose(out=dtp[:], in_=dt3[:],
                                        identity=id_sb[0:12, 0:12])
                    dt = epool.tile([D, 12], F32, tag="dt")
                    nc.scalar.copy(out=dt[:], in_=dtp[:])
                    rd = epool.tile([D, 12], F32, tag="rd")
                    nc.vector.reciprocal(out=rd[:], in_=dt[:])
                    cs = epool.tile([D, 12], F32, tag="cs")
                    nc.scalar.activation(out=cs[:], in_=dt[:],
                                         func=mybir.ActivationFunctionType.Sin,
                                         scale=float(np.pi / CUTOFF),
                                         bias=nhp_sb[:, 0:1])
                    env = epool.tile([D, 12], F32, tag="env")
                    nc.vector.tensor_scalar(out=env[:], in0=cs[:],
                                            scalar1=-0.5, scalar2=0.5,
                                            op0=mybir.AluOpType.mult,
                                            op1=mybir.AluOpType.add)
                    scl = epool.tile([D, 12], F32, tag="scl")
                    nc.vector.tensor_tensor(out=scl[:], in0=env[:], in1=rd[:],
                                            op=mybir.AluOpType.mult)
                    # u[32j+n, 128k+e] = coef_n * d[(3k+j)*128+e] via K=3 matmuls
                    u = bigp.tile([D, 512], F32, tag="mm512")
                    for k in range(4):
                        nc.tensor.matmul(out=u[:, k * D:(k + 1) * D],
                                         lhsT=cm_sb[:, k * D:(k + 1) * D],
                                         rhs=dt3[:],
                                         start=True, stop=True)
                    kf = epool.tile([D, 512], F32, tag="kf")
                    nc.vector.tensor_scalar(out=kf[:], in0=u[:],
                                            scalar1=MAGIC, scalar2=MAGIC,
                                            op0=mybir.AluOpType.add,
                                            op1=mybir.AluOpType.subtract)
                    v = epool.tile([D, 512], F32, tag="v")
                    nc.vector.tensor_tensor(out=v[:], in0=u[:], in1=kf[:],
                                            op=mybir.AluOpType.subtract)
                    db = epool.tile([D, 512], F32, tag="db")
                    # full-tile copy seeds the raw-d rows (32j+20); Sin then
                    # overwrites rows 32j..32j+19 (ops must be 32-part aligned)
                    nc.vector.tensor_copy(out=db[:], in_=u[:])
                    for j in range(3):
                        nc.scalar.activation(
                            out=db[32 * j:32 * j + NB, :],
                            in_=v[32 * j:32 * j + NB, :],
                            func=mybir.ActivationFunctionType.Sin,
                            scale=float(2 * np.pi))
                    for t in range(12):
                        k, j = t // 3, t % 3
                        pw = smallp.tile([D, D], F32, tag="mm128")
                        nc.tensor.matmul(
                            out=pw[:],
                            lhsT=db[32 * j:32 * j + NB + 1, k * D:(k + 1) * D],
                            rhs=wext_sb[32 * j:32 * j + NB + 1, :],
                            start=True, stop=True)
                        ws = epool.tile([D, D], F32, tag="ws")
                        nc.scalar.activation(
                            out=ws[:], in_=pw[:],
                            func=mybir.ActivationFunctionType.Copy,
                            scale=scl[:, t:t + 1])
                        slot = c2 * 12 + t
                        nc.vector.tensor_tensor(
                            out=out_sb[:, slot, :], in0=ws[:],
                            in1=phi[:, slot, :], op=mybir.AluOpType.mult)
                nc.sync.dma_start(
                    out=out_dev[g * GCH:(g + 1) * GCH, :].rearrange(
                        "(s p) f -> p s f", p=D),
                    in_=out_sb[:])
    nc.finalize()
    return nc


_NC_CACHE = {}


def kernel(s_j, dist, nbrs, W1, b1, W2, b2, W_rbf, b_rbf):
    s_j = np.asarray(s_j, dtype=np.float32)
    dist = np.asarray(dist, dtype=np.float32)
    j_idx = np.asarray(nbrs)[:, 1].astype(np.int32)

    core = j_idx // NPC
    order = np.argsort(core, kind="stable")
    counts = np.bincount(core, minlength=N_CORES)
    n_g = int((counts.max() + GCH - 1) // GCH)
    E_pad = n_g * GCH

    w21 = np.concatenate([np.asarray(W_rbf, np.float32),
                          np.asarray(b_rbf, np.float32)[None, :]], axis=0)
    wext = np.zeros((D, D), dtype=np.float32)
    for qj in range(3):
        wext[32 * qj:32 * qj + NB + 1] = w21
    # cmat[t, 128k+32j+n] = delta(t, 3k+j) * coef_n
    # coef_n = (n+1)/10 (n<20), 1.0 (n=20, the raw-d row), 0 else
    coef = np.zeros(32, dtype=np.float32)
    coef[:NB] = (np.arange(NB) + 1) / 10.0
    coef[NB] = 1.0
    cmat = np.zeros((12, 512), dtype=np.float32)
    for k in range(4):
        for j in range(3):
            cmat[3 * k + j, 128 * k + 32 * j:128 * k + 32 * j + 32] = coef
    pk32 = np.zeros((D, 896), dtype=np.float32)
    pk32[:, 0:D] = wext
    pk32[:, D:2 * D] = np.eye(D, dtype=np.float32)
    pk32[0:12, 2 * D:2 * D + 512] = cmat
    pk32[:, 768] = np.asarray(b1, np.float32)
    pk32[:, 769] = -np.pi / 2
    pk16 = np.zeros((D, 512), dtype=np.float16)
    pk16[:, 0:D] = np.asarray(W1, np.float32).astype(np.float16)
    pk16[:, D:2 * D] = np.asarray(W2, np.float32).astype(np.float16)
    pk16[0, 2 * D:3 * D] = 1.0
    pk16[0, 3 * D:4 * D] = np.asarray(b2, np.float32).astype(np.float16)
    pk32_rows = pk32.view(np.int32).reshape(-1, D)
    pk16_rows = pk16.view(np.int32).reshape(-1, D)

    NR_E = E_pad // D
    S0 = 2 * NR_E
    P0 = S0 + 3328
    Q0 = P0 + 896
    R_total = Q0 + 256

    starts = np.zeros(N_CORES + 1, dtype=np.int64)
    starts[1:] = np.cumsum(counts)
    in_maps = []
    for c in range(N_CORES):
        sel = order[starts[c]:starts[c + 1]]
        n_c = counts[c]
        blob = np.zeros((R_total, D), dtype=np.int32)
        idx_pad = blob[0:NR_E].reshape(-1)
        idx_pad[:n_c] = j_idx[sel] - c * NPC
        dist_pad = blob[NR_E:S0].reshape(-1).view(np.float32)
        dist_pad[:] = 1.0
        dist_pad[:n_c] = dist[sel]
        s_T = np.zeros((D, NPT), dtype=np.float16)
        s_T[:, :NPC] = s_j[c * NPC:(c + 1) * NPC].T
        blob[S0:P0] = s_T.view(np.int32).reshape(-1, D)
        blob[P0:Q0] = pk32_rows
        blob[Q0:R_total] = pk16_rows
        in_maps.append({"blob": blob})

    if n_g not in _NC_CACHE:
        _NC_CACHE[n_g] = build_nc(n_g)
    nc = _NC_CACHE[n_g]

    res = run_bass_kernel_spmd(nc, in_maps, list(range(N_CORES)))
    valid = np.concatenate(
        [res.results[c]["out_dev"][:counts[c]] for c in range(N_CORES)], axis=0)
    out = np.empty((N_EDGES, D), dtype=np.float32)
    out[order] = valid.astype(np.float32)
    return out


# revision 38
# speedup vs baseline: 1.1870x; 1.0364x over previous
"""Trainium2 Bass kernel for InvariantMessage GNN message passing.

out[e, :] = (MLP(s_j)[nbrs[e,1]]) * ((rbf(dist[e]) @ W_rbf + b_rbf) * env(dist[e]))

The axon tunnel (~40 MB/s D2H uncompressed, ~100 MB/s H2D on compressible
data, ~20 ms latency per array shard) dominates wall time -- on-device
compute is ~0.1 s -- so the design minimizes host<->device bytes and
transfer count:
- Nodes are split 6250/core; each EDGE is assigned to the core that owns its
  gathered node nbrs[e,1] (host knows the indices), so every gather is
  core-local (no collectives) and s_j is uploaded exactly once across the
  fleet (fp16, 1.7 MB/core) instead of replicated x8 in fp32.
- ALL per-core inputs (idx, dist, s_j^T fp16, packed fp32/fp16 constants)
  ride in ONE int32 blob (~3.1 MB/core); device-side slices use
  rearrange+bitcast. Fewer arrays = fewer per-shard latency hits.
- Each core runs the MLP (fp16 weights/activations, fp32 PSUM; bias b2 is
  added by an accumulating ones-outer-product matmul, giving [node, feat]
  orientation without a transpose) on its 6250 nodes into an Internal-DRAM
  table -- Internal kind avoids uploading a zero table. Then per 3072-edge
  chunk phi rows are fetched with indirect_dma_start (128 rows/instruction,
  the HW-validated semantics; int16 dma_gather hung the device).
- dist is uploaded once ([12,128] rows per 1536-edge block). The quadrant-
  packed sin-argument tile u[32j+n, 128k+e] = coef_n * d is built on device
  by K=12 matmuls against a selection-coefficient matrix (coef_n=(n+1)/10
  for n<20, 1.0 for n=20, the raw-d row the K=21 rbf matmul uses for its
  b_rbf term), range-reduced to [-.5,.5] with the fp32 magic-number trick,
  evaluated by ScalarE Sin (rows seeded by a full-tile DVE copy since engine
  ops must be 32-partition aligned), then a K=21 matmul against
  [W_rbf; b_rbf] gives w*d in PSUM; scaling by env/d (from a tensor-engine
  transpose of the dist rows) and the gathered phi finishes the edge.
- Output is fp16 (halves both the donated-zero upload run_bass_via_pjrt
  forces and the uncompressible fetch) and upcast on host; worst-case
  per-element error ~0.3%, observed 6e-4 global-relative.
Warm-call budget: ~5.1 s output fetch + ~2.0 s donated-zero upload +
~0.6 s blob upload + ~0.45 s executable load (jax persistent cache) +
~0.12 s execute = ~8.2 s, vs 28.3 s for the previous kernel.
"""
import sys

sys.path.insert(0, "/opt/trn_rl_repo")

import numpy as np

try:  # cache XLA executables across calls (the runner re-jits every call)
    import jax
    jax.config.update("jax_compilation_cache_dir", "/tmp/jax_comp_cache")
    jax.config.update("jax_persistent_cache_min_entry_size_bytes", 0)
    jax.config.update("jax_persistent_cache_min_compile_time_secs", 0)
except Exception:
    pass

import concourse.tile as tile
from concourse import bass, bacc, mybir
from concourse.bass_utils import run_bass_kernel_spmd

F32 = mybir.dt.float32
F16 = mybir.dt.float16
I32 = mybir.dt.int32

N_CORES = 8
N_ATOMS = 50000
N_EDGES = 800000
D = 128
NB = 20
CUTOFF = 5.0
MAGIC = float(np.float32(1.5 * 2**23))

NPC = N_ATOMS // N_CORES          # 6250 nodes per core
NPT = 6656                        # table rows = 13*512 (>= NPC)
NCH_NODE = NPT // 512             # 13 node MLP chunks
GCH = 3072                        # edges per gather/output chunk


def build_nc(n_gchunks):
    nc = bacc.Bacc(None, target_bir_lowering=False)
    E_pad = n_gchunks * GCH
    NR16 = E_pad // 256
    NR_E = E_pad // D

    # All inputs ride in ONE int32 blob (the axon tunnel charges ~20 ms
    # latency per array shard, so fewer arrays = faster upload), with no
    # replicated or padded constant data:
    #   rows [0, NR16)        idx as packed int16 (local ids < 6250)
    #   rows [D0, D0+NR_E)    dist fp32 bits, row r = d[128r : 128r+128]
    #   rows [S0, S0+3328)    s_jT fp16 [128, 6656]; row S0+26p+k holds
    #                         int32 cols 128k:128k+128 of partition p
    #   const section at C0 (212 rows):
    #     +0   [21,128] f32  [W_rbf; b_rbf] (device replicates x3 quadrants)
    #     +21  [12,12]  f32  transpose identity (cols 0:12)
    #     +33  [12,512] f32  cmat, row p*4+k layout
    #     +81  [2,128]  f32  b1 row | -pi/2 row
    #     +83  [128,128]f16  W1 (int32 cols 0:64) | W2 (cols 64:128)
    #     +211 [1,128]  f16  b2 (int32 cols 0:64)
    D0 = NR16
    S0 = D0 + NR_E
    C0 = S0 + 3328
    R_total = C0 + 212
    blob = nc.dram_tensor("blob", [R_total, D], I32, kind="ExternalInput")
    out_dev = nc.dram_tensor("out_dev", [E_pad, D], F16, kind="ExternalOutput")

    inv = nc.dram_tensor("inv", [NPT, D], F32, kind="Internal")

    s_ap = blob[S0:S0 + 3328, :].rearrange("(p k) e -> p (k e)", p=D)

    with tile.TileContext(nc) as tc:
        with tc.tile_pool(name="const", bufs=1) as cpool, \
             tc.tile_pool(name="mlp", bufs=3) as mpool, \
             tc.tile_pool(name="bigp", bufs=2, space="PSUM") as bigp, \
             tc.tile_pool(name="smallp", bufs=4, space="PSUM") as smallp, \
             tc.tile_pool(name="dtpp", bufs=2, space="PSUM") as dtpp, \
             tc.tile_pool(name="edge", bufs=3) as epool, \
             tc.tile_pool(name="big", bufs=2) as bpool:

            wext_sb = cpool.tile([D, D], F32)
            for j in range(3):
                nc.sync.dma_start(out=wext_sb[32 * j:32 * j + NB + 1, :],
                                  in_=blob[C0:C0 + 21, :].bitcast(F32))
            id_sb = cpool.tile([12, 12], F32)
            nc.sync.dma_start(out=id_sb[:],
                              in_=blob[C0 + 21:C0 + 33, 0:12].bitcast(F32))
            cm_sb = cpool.tile([12, 512], F32)
            nc.sync.dma_start(out=cm_sb[:],
                              in_=blob[C0 + 33:C0 + 81, :].rearrange(
                                  "(p k) e -> p (k e)", p=12).bitcast(F32))
            bv = cpool.tile([D, 2], F32)
            nc.sync.dma_start(out=bv[:],
                              in_=blob[C0 + 81:C0 + 83, :].rearrange(
                                  "o p -> p o").bitcast(F32))
            b1_sb = bv[:, 0:1]
            nhp_sb = bv[:, 1:2]
            w1_sb = cpool.tile([D, D], F16)
            nc.sync.dma_start(out=w1_sb[:],
                              in_=blob[C0 + 83:C0 + 211, 0:64].bitcast(F16))
            w2_sb = cpool.tile([D, D], F16)
            nc.sync.dma_start(out=w2_sb[:],
                              in_=blob[C0 + 83:C0 + 211, 64:128].bitcast(F16))
            ones_sb = cpool.tile([1, D], F16)
            nc.vector.memset(ones_sb[:], 1.0)
            b2r_sb = cpool.tile([1, D], F16)
            nc.sync.dma_start(out=b2r_sb[:],
                              in_=blob[C0 + 211:C0 + 212, 0:64].bitcast(F16))

            # ---- Phase 1: node MLP -> inv table (node-major rows) ----
            for i in range(NCH_NODE):
                s_t = mpool.tile([D, 512], F16, tag="s")
                nc.sync.dma_start(
                    out=s_t[:],
                    in_=s_ap[:, i * 256:(i + 1) * 256].bitcast(F16))
                ph = bigp.tile([D, 512], F32, tag="mm512")
                nc.tensor.matmul(out=ph[:], lhsT=w1_sb[:], rhs=s_t[:],
                                 start=True, stop=True)
                h_t = mpool.tile([D, 512], F16, tag="h")
                nc.scalar.activation(out=h_t[:], in_=ph[:],
                                     func=mybir.ActivationFunctionType.Silu,
                                     bias=b1_sb[:, 0:1], scale=1.0)
                for b in range(4):
                    pt = smallp.tile([D, D], F32, tag="mm128")
                    nc.tensor.matmul(out=pt[:],
                                     lhsT=h_t[:, b * D:(b + 1) * D],
                                     rhs=w2_sb[:], start=True, stop=False)
                    nc.tensor.matmul(out=pt[:], lhsT=ones_sb[:],
                                     rhs=b2r_sb[:], start=False, stop=True)
                    ot = mpool.tile([D, D], F32, tag="ot")
                    nc.scalar.copy(out=ot[:], in_=pt[:])
                    n0 = i * 512 + b * D
                    nc.sync.dma_start(out=inv[n0:n0 + D, :], in_=ot[:])

            # ---- Phase 2: edges ----
            for g in range(n_gchunks):
                ix16 = epool.tile([D, GCH // D], mybir.dt.int16, tag="ix16")
                nc.sync.dma_start(
                    out=ix16[:],
                    in_=blob[g * (GCH // 256):(g + 1) * (GCH // 256),
                             :].bitcast(mybir.dt.int16).rearrange(
                                 "r (s2 p) -> p (r s2)", p=D))
                ix = epool.tile([D, GCH // D], I32, tag="ix")
                nc.vector.tensor_copy(out=ix[:], in_=ix16[:])
                phi = bpool.tile([D, GCH // D, D], F32, tag="phi")
                for s in range(GCH // D):
                    nc.gpsimd.indirect_dma_start(
                        out=phi[:, s, :], out_offset=None, in_=inv[:],
                        in_offset=bass.IndirectOffsetOnAxis(
                            ap=ix[:, s:s + 1], axis=0))
                out_sb = bpool.tile([D, GCH // D, D], F16, tag="osb")
                for c2 in range(GCH // 1536):
                    blk = g * 2 + c2
                    # dist rows for this 1536-edge block: [12, 128]
                    dt3 = epool.tile([12, D], F32, tag="dt3")
                    nc.sync.dma_start(
                        out=dt3[:],
                        in_=blob[D0 + blk * 12:D0 + (blk + 1) * 12,
                                 :].bitcast(F32))
                    # transpose -> [128, 12] per-partition dist scalars
                    dtp = dtpp.tile([D, 12], F32, tag="dtp")
                    nc.tensor.transpose(out=dtp[:], in_=dt3[:],
                                        identity=id_sb[:])
                    dt = epool.tile([D, 12], F32, tag="dt")
                    nc.scalar.copy(out=dt[:], in_=dtp[:])
                    rd = epool.tile([D, 12], F32, tag="rd")
                    nc.vector.reciprocal(out=rd[:], in_=dt[:])
                    cs = epool.tile([D, 12], F32, tag="cs")
                    nc.scalar.activation(out=cs[:], in_=dt[:],
                                         func=mybir.ActivationFunctionType.Sin,
                                         scale=float(np.pi / CUTOFF),
                                         bias=nhp_sb[:, 0:1])
                    env = epool.tile([D, 12], F32, tag="env")
                    nc.vector.tensor_scalar(out=env[:], in0=cs[:],
                                            scalar1=-0.5, scalar2=0.5,
                                            op0=mybir.AluOpType.mult,
                                            op1=mybir.AluOpType.add)
                    scl = epool.tile([D, 12], F32, tag="scl")
                    nc.vector.tensor_tensor(out=scl[:], in0=env[:], in1=rd[:],
                                            op=mybir.AluOpType.mult)
                    # u[32j+n, 128k+e] = coef_n * d[(3k+j)*128+e] via K=3 matmuls
                    u = bigp.tile([D, 512], F32, tag="mm512")
                    for k in range(4):
                        nc.tensor.matmul(out=u[:, k * D:(k + 1) * D],
                                         lhsT=cm_sb[:, k * D:(k + 1) * D],
                                         rhs=dt3[:],
                                         start=True, stop=True)
                    kf = epool.tile([D, 512], F32, tag="kf")
                    nc.vector.tensor_scalar(out=kf[:], in0=u[:],
                                            scalar1=MAGIC, scalar2=MAGIC,
                                            op0=mybir.AluOpType.add,
                                            op1=mybir.AluOpType.subtract)
                    v = epool.tile([D, 512], F32, tag="v")
                    nc.vector.tensor_tensor(out=v[:], in0=u[:], in1=kf[:],
                                            op=mybir.AluOpType.subtract)
                    db = epool.tile([D, 512], F32, tag="db")
                    # full-tile copy seeds the raw-d rows (32j+20); Sin then
                    # overwrites rows 32j..32j+19 (ops must be 32-part aligned)
                    nc.vector.tensor_copy(out=db[:], in_=u[:])
                    for j in range(3):
                        nc.scalar.activation(
                            out=db[32 * j:32 * j + NB, :],
                            in_=v[32 * j:32 * j + NB, :],
                            func=mybir.ActivationFunctionType.Sin,
                            scale=float(2 * np.pi))
                    for t in range(12):
                        k, j = t // 3, t % 3
                        pw = smallp.tile([D, D], F32, tag="mm128")
                        nc.tensor.matmul(
                            out=pw[:],
                            lhsT=db[32 * j:32 * j + NB + 1, k * D:(k + 1) * D],
                            rhs=wext_sb[32 * j:32 * j + NB + 1, :],
                            start=True, stop=True)
                        ws = epool.tile([D, D], F32, tag="ws")
                        nc.scalar.activation(
                            out=ws[:], in_=pw[:],
                            func=mybir.ActivationFunctionType.Copy,
                            scale=scl[:, t:t + 1])
                        slot = c2 * 12 + t
                        nc.vector.tensor_tensor(
                            out=out_sb[:, slot, :], in0=ws[:],
                            in1=phi[:, slot, :], op=mybir.AluOpType.mult)
                nc.sync.dma_start(
                    out=out_dev[g * GCH:(g + 1) * GCH, :].rearrange(
                        "(s p) f -> p s f", p=D),
                    in_=out_sb[:])
    nc.finalize()
    return nc


_NC_CACHE = {}


def kernel(s_j, dist, nbrs, W1, b1, W2, b2, W_rbf, b_rbf):
    s_j = np.asarray(s_j, dtype=np.float32)
    dist = np.asarray(dist, dtype=np.float32)
    j_idx = np.asarray(nbrs)[:, 1].astype(np.int32)

    core = j_idx // NPC
    order = np.argsort(core, kind="stable")
    counts = np.bincount(core, minlength=N_CORES)
    n_g = int((counts.max() + GCH - 1) // GCH)
    E_pad = n_g * GCH

    w21 = np.ascontiguousarray(
        np.concatenate([np.asarray(W_rbf, np.float32),
                        np.asarray(b_rbf, np.float32)[None, :]], axis=0))
    # cmat[t, 128k+32j+n] = delta(t, 3k+j) * coef_n
    # coef_n = (n+1)/10 (n<20), 1.0 (n=20, the raw-d row), 0 else
    coef = np.zeros(32, dtype=np.float32)
    coef[:NB] = (np.arange(NB) + 1) / 10.0
    coef[NB] = 1.0
    cmat = np.zeros((12, 512), dtype=np.float32)
    for k in range(4):
        for j in range(3):
            cmat[3 * k + j, 128 * k + 32 * j:128 * k + 32 * j + 32] = coef
    crows = np.zeros((212, D), dtype=np.int32)
    crows[0:21] = w21.view(np.int32)
    id12 = np.zeros((12, D), dtype=np.float32)
    id12[:, 0:12] = np.eye(12, dtype=np.float32)
    crows[21:33] = id12.view(np.int32)
    crows[33:81] = cmat.view(np.int32).reshape(-1, D)
    crows[81] = np.asarray(b1, np.float32).view(np.int32)
    crows[82] = np.full(D, -np.pi / 2, np.float32).view(np.int32)
    crows[83:211, 0:64] = np.asarray(W1, np.float32).astype(
        np.float16).view(np.int32)
    crows[83:211, 64:128] = np.asarray(W2, np.float32).astype(
        np.float16).view(np.int32)
    crows[211, 0:64] = np.asarray(b2, np.float32).astype(
        np.float16).reshape(1, D).view(np.int32)

    NR16 = E_pad // 256
    NR_E = E_pad // D
    D0 = NR16
    S0 = D0 + NR_E
    C0 = S0 + 3328
    R_total = C0 + 212

    starts = np.zeros(N_CORES + 1, dtype=np.int64)
    starts[1:] = np.cumsum(counts)
    in_maps = []
    for c in range(N_CORES):
        sel = order[starts[c]:starts[c + 1]]
        n_c = counts[c]
        blob = np.zeros((R_total, D), dtype=np.int32)
        idx_pad = blob[0:NR16].reshape(-1).view(np.int16)
        idx_pad[:n_c] = (j_idx[sel] - c * NPC).astype(np.int16)
        dist_pad = blob[D0:S0].reshape(-1).view(np.float32)
        dist_pad[:] = 1.0
        dist_pad[:n_c] = dist[sel]
        s_T = np.zeros((D, NPT), dtype=np.float16)
        s_T[:, :NPC] = s_j[c * NPC:(c + 1) * NPC].T
        blob[S0:C0] = s_T.view(np.int32).reshape(-1, D)
        blob[C0:R_total] = crows
        in_maps.append({"blob": blob})

    if n_g not in _NC_CACHE:
        _NC_CACHE[n_g] = build_nc(n_g)
    nc = _NC_CACHE[n_g]

    res = run_bass_kernel_spmd(nc, in_maps, list(range(N_CORES)))
    valid = np.concatenate(
        [res.results[c]["out_dev"][:counts[c]] for c in range(N_CORES)], axis=0)
    out = np.empty((N_EDGES, D), dtype=np.float32)
    out[order] = valid.astype(np.float32)
    return out
